# revision 13
# baseline (speedup 1.0000x reference)
"""Trainium2 Bass kernel for the WENO5 convection-diffusion-dispersion RHS.

dudt = -ALPHA * WENO_Godunov_flux_divergence(0.5 u^2) + BETA*u_xx - GAMMA*u_xxx
(periodic), for u of shape [4096, 8192] fp32.

Sharding: data-parallel over the batch axis across 8 NeuronCores (512 rows
per core).  On-chip layout: batch on the 128 SBUF partitions, the spatial
axis on the free dimension so every stencil shift is a free AP offset.

Math restructuring (verified against the reference algebra):
  G[m]   = U[m+1]-U[m]
  d2[m]  = G[m]-G[m-1]          (= U[m-1]-2U[m]+U[m+1])
  r[m]   = 3G[m]-G[m-1]         (= U[m-1]-4U[m]+3U[m+1])
  l[m]   = G[m]-3G[m-1]         (= 3U[m-1]-4U[m]+U[m+1])
  d[m]   = -(G[m]+G[m-1])       (= U[m-1]-U[m+1])
  beta_R = c13*d2^2 + 0.25 r^2 ; beta_C = c13*d2^2 + 0.25 d^2
  beta_L = c13*d2^2 + 0.25 l^2
  Qx[m]  = (s*(beta_x + EPS))^2            <- one fused custom DVE op each
  um(i) uses (q0,q1,q2) = (QR[i-2],QC[i-1],QL[i]),
  up(i) uses (q0,q1,q2) = (QL[i+1],QC[i],QR[i-1]);
  multiplying num/den by q0*q1*q2 gives products
    P_RL[m]=QR[m]*QL[m+2], P_RC[m]=QR[m]*QC[m+1], P_CL[m]=QC[m]*QL[m+1]
  shared between um and up.  Candidate polynomials (cell-centric, /6):
    PA = U + d2/3 + 1.5G[m],  PAr = U + d2/3 - 1.5G[m-1]
    PB = U - d2/6 + 0.5G[m],  PBr = U - d2/6 - 0.5G[m-1]
  um(i) = 10*Nm*(1/Dm), Nm = .1*P_CL[i-1]*PA[i-2] + .6*P_RL[i-2]*PB[i-1]
                             + .3*P_RC[i-2]*PBr[i]
          Dm = .5*P_RC[i-2] + (P_CL[i-1]/6 + P_RL[i-2])   (=den/0.6)
  up(i) analogous with (P_RC[i-1],PAr[i+1]) / (P_RL[i-1],PBr[i]) /
  (P_CL[i],PB[i-1]).
  fhat(i) = 0.5*max(relu(um)^2, min(up,0)^2); flux fused as
    F'[i] = (50*ALPHA/DX) * max(sq(relu(Nm*rm)), sq(min(Np*rp,0)))
  FDM part carried at c2-scale: d2s = c2*d2, A3 = (c3/c2)*(d2s[j+1]-d2s[j-1])
  + d2s[j];  out[j] = (F'[j]-F'[j+1]) + A3[j].

EPS is raised from 1e-16 to 1e-6 inside the WENO weights only: the weights
are identical to fp32 rounding except on ~1e-6 of cells, where the induced
flux error is ~1e-6 relative to the (u_xxx-dominated) output. This keeps the
q-products inside fp32 dynamic range.
"""

import math

import numpy as np

import concourse.bass as bass
import concourse.bacc as bacc
import concourse.mybir as mybir
import concourse.tile as tile
from concourse import dve_ops
from concourse.bass_utils import run_bass_kernel_spmd
from concourse.dve_spec import (
    C0,
    C1,
    C2,
    Spec,
    Src0,
    Src1,
    Zero,
    lower,
    minn,
    relu,
    sq,
)
from concourse.dve_uop import DveOpSpec

# ---- problem constants -----------------------------------------------------
B, NX = 4096, 8192
N_CORES = 8
ROWS_PER_CORE = B // N_CORES  # 512
L = 16.0
DX = L / NX
ALPHA, BETA, GAMMA = 3.0, 0.1, 1.0
EPS_K = 1e-6  # WENO regulariser used on-chip (reference uses 1e-16; see above)
C13 = 13.0 / 12.0
SQ_S = math.sqrt(1e3)  # sqrt of inner q-scale s
C2_FDM = BETA / DX / DX  # 26214.4
C3_FDM = -GAMMA / (2.0 * DX**3)  # -67108864.0
# um = Nm/(0.6*Dm) -> relu(um)^2 = relu(Nm*rm)^2/0.36; fhat = 0.5*max(...)
FLUXK = 0.5 * ALPHA / (0.36 * DX)  # scale on the fused max() flux terms

F32 = mybir.dt.float32
ADD = mybir.AluOpType.add
SUB = mybir.AluOpType.subtract
MUL = mybir.AluOpType.mult

# ---- custom fused DVE ops --------------------------------------------------
_REGISTERED = {}


def _register_dve(name, spec, subdim=False):
    """Register a custom DVE op in the dve_ops tables, computing its sha."""
    if name in _REGISTERED:
        return _REGISTERED[name]
    from concourse.dve_spec import _has_src1 as has_src1

    opcode = dve_ops._CUSTOM_DVE_ROW_BASE + len(dve_ops.OPS)
    shas = {}
    for ver in ("v3", "v4"):
        try:
            compiled = DveOpSpec(
                name=name,
                opcode=opcode,
                uops=lower(spec, ver=ver),
                rd1_en=has_src1(spec),
            )
            shas[ver] = compiled.sha(ver)
        except Exception:
            pass
    op = dve_ops.DveOp(name, spec, subdim=subdim, uops_sha=shas)
    dve_ops.OPS.append(op)
    dve_ops._SUB_OPCODE_FOR_NAME[name] = opcode
    dve_ops.CUSTOM_DVE_SPECS[name] = spec
    _REGISTERED[name] = op
    return op


def _q_specs():
    # scaled smoothness beta~ = s*beta, fused per flavour; the final
    # (beta~+eps~)^2 runs on the ScalarEngine as Square(x + eps~).
    # Src0 = G[m], Src1 = G[m-1].  (No Python literals in Spec bodies:
    # 3*S0-S1 == (S0-S1)+(S0+S0), S0-3*S1 == (S0-S1)-(S1+S1).)
    t = Src0 - Src1
    ca = sq(t * C0)  # c13*s*d2^2
    br = ca + sq((t + (Src0 + Src0)) * C1)
    bc = ca + sq((Src0 + Src1) * C1)
    bl = ca + sq((t - (Src1 + Src1)) * C1)
    return br, bc, bl


_BR_BODY, _BC_BODY, _BL_BODY = _q_specs()
OP_BR = _register_dve("ANT_WENO_BR", Spec(body=_BR_BODY))
OP_BC = _register_dve("ANT_WENO_BC", Spec(body=_BC_BODY))
OP_BL = _register_dve("ANT_WENO_BL", Spec(body=_BL_BODY))
# d2s = C0*(Src0-Src1)
OP_D2S = _register_dve("ANT_D2SCALE", Spec(body=(Src0 - Src1) * C0))
# C0*relu(Src0*Src1)^2  and  C0*min(Src0*Src1,0)^2
OP_RELSQ = _register_dve("ANT_RELSQS", Spec(body=sq(relu(Src0 * Src1)) * C0))
OP_MINSQ = _register_dve("ANT_MINSQS", Spec(body=sq(minn(Src0 * Src1, Zero)) * C0))


# ---- kernel body -----------------------------------------------------------
W = 1024  # spatial tile width (free axis)
# Total-order instruction chain: this walrus build rejects >1 sync wait on
# compute instructions; the chain guarantees exactly one.
LINEARIZE = False


def _emit_tile(nc, pool, u_d, o_d, rb, ct):
    """Emit one [128 x W] output tile (row block rb, column tile ct)."""
    vec = nc.vector
    r0, r1 = rb * 128, (rb + 1) * 128
    c0 = ct * W
    WU = W + 6  # U halo width: columns map m = -3 .. W+2

    def t(tag, width):
        return pool.tile([128, width], F32, tag=tag, name=f"{tag}_{rb}_{ct}")

    U = t("u", WU)
    # load with periodic wrap (halo 3 on both sides).  The TT ISA struct has
    # a single sync-wait slot, so a tile must not make its first consumer
    # wait on two DMAs: the small wrapped halo goes through a DVE copy (the
    # copy takes one DMA wait; program order on DVE covers it for the rest).
    lo, hi = c0 - 3, c0 + W + 3
    if lo < 0:
        Uh = t("uh", 3)
        nc.gpsimd.dma_start(Uh[:, :], u_d[r0:r1, NX + lo : NX])
        nc.gpsimd.dma_start(U[:, -lo : WU], u_d[r0:r1, 0 : hi])
        vec.tensor_copy(U[:, 0 : -lo], Uh[:, :])
    elif hi > NX:
        Uh = t("uh", 3)
        nc.gpsimd.dma_start(Uh[:, :], u_d[r0:r1, 0 : hi - NX])
        nc.gpsimd.dma_start(U[:, 0 : WU - (hi - NX)], u_d[r0:r1, lo:NX])
        vec.tensor_copy(U[:, WU - (hi - NX) : WU], Uh[:, :])
    else:
        nc.gpsimd.dma_start(U[:, :], u_d[r0:r1, lo:hi])

    # 01  G[m] = U[m+1]-U[m],  m = -3..W+1  (width W+5, col = m+3)
    G = t("g", W + 5)
    vec.tensor_sub(G[:, :], U[:, 1 : W + 6], U[:, 0 : W + 5])
    # 02  d2s[m] = c2*(G[m]-G[m-1]),  m = -2..W+1  (width W+4, col = m+2)
    d2s = t("d2s", W + 4)
    vec._custom_dve(
        OP_D2S, out=d2s[:, :], in0=G[:, 1 : W + 5], in1=G[:, 0 : W + 4], s0=C2_FDM
    )
    # 03-05  Q arrays, m = -2..W+1 (width W+4, col = m+2):
    # custom DVE computes beta~ = s*beta; ScalarE squares with +eps~ bias.
    qk0 = math.sqrt(C13) * SQ_S
    qk1 = 0.5 * SQ_S
    qk2 = EPS_K * 1e3  # eps~ = s*EPS_K
    QR = t("qr", W + 4)
    QC = t("qc", W + 4)
    QL = t("ql", W + 4)
    for op, dst, btag in ((OP_BR, QR, "br"), (OP_BC, QC, "bc"), (OP_BL, QL, "bl")):
        bt = t(btag, W + 4)
        vec._custom_dve(
            op,
            out=bt[:, :],
            in0=G[:, 1 : W + 5],
            in1=G[:, 0 : W + 4],
            s0=qk0,
            s1=qk1,
        )
        nc.scalar.activation(
            dst[:, :], bt[:, :], mybir.ActivationFunctionType.Square, bias=qk2
        )
    # 07-08  tA = U + d2s/(3 c2), tB = U - d2s/(6 c2)   (m = -2..W+1)
    tA = t("ta", W + 4)
    tB = t("tb", W + 4)
    vec.scalar_tensor_tensor(
        tA[:, :], d2s[:, :], 1.0 / (3 * C2_FDM), U[:, 1 : W + 5], MUL, ADD
    )
    vec.scalar_tensor_tensor(
        tB[:, :], d2s[:, :], -1.0 / (6 * C2_FDM), U[:, 1 : W + 5], MUL, ADD
    )
    # 09-12  candidates (m = -2..W+1, col = m+2)
    PA = t("pa", W + 4)
    PAr = t("par", W + 4)
    PB = t("pb", W + 4)
    PBr = t("pbr", W + 4)
    vec.scalar_tensor_tensor(PA[:, :], G[:, 1 : W + 5], 1.5, tA[:, :], MUL, ADD)
    vec.scalar_tensor_tensor(PAr[:, :], G[:, 0 : W + 4], -1.5, tA[:, :], MUL, ADD)
    vec.scalar_tensor_tensor(PB[:, :], G[:, 1 : W + 5], 0.5, tB[:, :], MUL, ADD)
    vec.scalar_tensor_tensor(PBr[:, :], G[:, 0 : W + 4], -0.5, tB[:, :], MUL, ADD)
    # 13-15  q-products (col = m+2)
    PRL = t("prl", W + 2)  # m = -2..W-1
    PRC = t("prc", W + 3)  # m = -2..W
    PCL = t("pcl", W + 3)
    vec.tensor_mul(PRL[:, :], QR[:, 0 : W + 2], QL[:, 2 : W + 4])
    vec.tensor_mul(PRC[:, :], QR[:, 0 : W + 3], QC[:, 1 : W + 4])
    vec.tensor_mul(PCL[:, :], QC[:, 0 : W + 3], QL[:, 1 : W + 4])
    # interfaces i = 0..W (width W+1);  P_* col(m)=m+2, cand col(m)=m+2
    WI = W + 1
    n1 = t("n1", WI)
    n2 = t("n2", WI)
    n12 = t("n12", WI)
    n3 = t("n3", WI)
    Nm = t("nm", WI)
    vec.scalar_tensor_tensor(n1[:, :], PCL[:, 1 : WI + 1], 0.1, PA[:, 0:WI], MUL, MUL)
    vec.scalar_tensor_tensor(n2[:, :], PRL[:, 0:WI], 0.6, PB[:, 1 : WI + 1], MUL, MUL)
    vec.tensor_add(n12[:, :], n1[:, :], n2[:, :])
    vec.scalar_tensor_tensor(n3[:, :], PRC[:, 0:WI], 0.3, PBr[:, 2 : WI + 2], MUL, MUL)
    vec.tensor_add(Nm[:, :], n12[:, :], n3[:, :])
    d1m = t("d1m", WI)
    Dm = t("dm", WI)
    vec.scalar_tensor_tensor(
        d1m[:, :], PCL[:, 1 : WI + 1], 1.0 / 6.0, PRL[:, 0:WI], MUL, ADD
    )
    vec.scalar_tensor_tensor(Dm[:, :], PRC[:, 0:WI], 0.5, d1m[:, :], MUL, ADD)
    n1p = t("n1p", WI)
    n2p = t("n2p", WI)
    n12p = t("n12p", WI)
    n3p = t("n3p", WI)
    Np = t("np", WI)
    vec.scalar_tensor_tensor(
        n1p[:, :], PRC[:, 1 : WI + 1], 0.1, PAr[:, 3 : WI + 3], MUL, MUL
    )
    vec.scalar_tensor_tensor(
        n2p[:, :], PRL[:, 1 : WI + 1], 0.6, PBr[:, 2 : WI + 2], MUL, MUL
    )
    vec.tensor_add(n12p[:, :], n1p[:, :], n2p[:, :])
    vec.scalar_tensor_tensor(
        n3p[:, :], PCL[:, 2 : WI + 2], 0.3, PB[:, 1 : WI + 1], MUL, MUL
    )
    vec.tensor_add(Np[:, :], n12p[:, :], n3p[:, :])
    d1p = t("d1p", WI)
    Dp = t("dp", WI)
    vec.scalar_tensor_tensor(
        d1p[:, :], PRC[:, 1 : WI + 1], 1.0 / 6.0, PRL[:, 1 : WI + 1], MUL, ADD
    )
    vec.scalar_tensor_tensor(Dp[:, :], PCL[:, 2 : WI + 2], 0.5, d1p[:, :], MUL, ADD)
    # 30-31 reciprocals (approx, ~18 bits — weight normalisation only)
    rm = t("rm", WI)
    rp = t("rp", WI)
    vec.reciprocal_approx_fast(out=rm[:, :], in_=Dm[:, :])
    vec.reciprocal_approx_fast(out=rp[:, :], in_=Dp[:, :])
    # 32-33 fused flux halves: FLUXK/100 * relu(10*Nm*rm)^2 etc.
    AM = t("am", WI)
    BM = t("bm", WI)
    vec._custom_dve(OP_RELSQ, out=AM[:, :], in0=Nm[:, :], in1=rm[:, :], s0=FLUXK)
    vec._custom_dve(OP_MINSQ, out=BM[:, :], in0=Np[:, :], in1=rp[:, :], s0=FLUXK)
    # 34 F'[i] = max(AM,BM)
    F = t("f", WI)
    vec.tensor_max(F[:, :], AM[:, :], BM[:, :])
    # FDM tail (output cells j = 0..W-1)
    A2s = t("a2s", W)
    A3f = t("a3f", W)
    A1 = t("a1", W)
    OUT = t("out", W)
    vec.tensor_sub(A2s[:, :], d2s[:, 3 : W + 3], d2s[:, 1 : W + 1])
    vec.scalar_tensor_tensor(
        A3f[:, :], A2s[:, :], C3_FDM / C2_FDM, d2s[:, 2 : W + 2], MUL, ADD
    )
    vec.tensor_sub(A1[:, :], F[:, 0:W], F[:, 1 : W + 1])
    vec.tensor_add(OUT[:, :], A1[:, :], A3f[:, :])
    nc.gpsimd.dma_start(o_d[r0:r1, c0 : c0 + W], OUT[:, :])


def _build_nc():
    nc = bacc.Bacc("TRN2", target_bir_lowering=False, debug=False)
    # const AP for the ScalarE Square bias (eps~), same pattern as Bass init
    eps_val = EPS_K * 1e3
    eps_t = nc.alloc_sbuf_tensor("const-float32-weno-eps", [128, 1], F32)
    nc.gpsimd.memset(eps_t.ap(), eps_val)
    nc.const_aps.aps[(F32, eps_val)] = eps_t.ap()
    nc.all_engine_barrier()
    u_d = nc.dram_tensor("u", [ROWS_PER_CORE, NX], F32, kind="ExternalInput")
    o_d = nc.dram_tensor("out", [ROWS_PER_CORE, NX], F32, kind="ExternalOutput")
    with tile.TileContext(nc, linearize=LINEARIZE) as tc:
        with tc.tile_pool(name="main", bufs=1) as pool:
            for rb in range(ROWS_PER_CORE // 128):
                for ct in range(NX // W):
                    _emit_tile(nc, pool, u_d, o_d, rb, ct)
    nc.compile()
    return nc


_NC = None


def _get_nc():
    global _NC
    if _NC is None:
        _NC = _build_nc()
    return _NC


def _execute(u, trace=False):
    nc = _get_nc()
    u = np.ascontiguousarray(np.asarray(u, dtype=np.float32))
    in_maps = [
        {"u": u[i * ROWS_PER_CORE : (i + 1) * ROWS_PER_CORE]} for i in range(N_CORES)
    ]
    res = run_bass_kernel_spmd(nc, in_maps, list(range(N_CORES)), trace=trace)
    out = np.concatenate([res.results[i]["out"] for i in range(N_CORES)], axis=0)
    return out, res


def kernel(u, t=None, **_ignored):
    out, _ = _execute(u, trace=False)
    return out


# revision 16
# speedup vs baseline: 1.0301x; 1.0301x over previous
"""Trainium2 Bass kernel for the WENO5 convection-diffusion-dispersion RHS.

dudt = -ALPHA * WENO_Godunov_flux_divergence(0.5 u^2) + BETA*u_xx - GAMMA*u_xxx
(periodic), for u of shape [4096, 8192] fp32.

Sharding: data-parallel over the batch axis across 8 NeuronCores (512 rows
per core).  On-chip layout: batch on the 128 SBUF partitions, the spatial
axis on the free dimension so every stencil shift is a free AP offset.

Math restructuring (verified against the reference algebra):
  G[m]   = U[m+1]-U[m]
  d2[m]  = G[m]-G[m-1]          (= U[m-1]-2U[m]+U[m+1])
  r[m]   = 3G[m]-G[m-1]         (= U[m-1]-4U[m]+3U[m+1])
  l[m]   = G[m]-3G[m-1]         (= 3U[m-1]-4U[m]+U[m+1])
  d[m]   = -(G[m]+G[m-1])       (= U[m-1]-U[m+1])
  beta_R = c13*d2^2 + 0.25 r^2 ; beta_C = c13*d2^2 + 0.25 d^2
  beta_L = c13*d2^2 + 0.25 l^2
  Qx[m]  = (s*(beta_x + EPS))^2            <- one fused custom DVE op each
  um(i) uses (q0,q1,q2) = (QR[i-2],QC[i-1],QL[i]),
  up(i) uses (q0,q1,q2) = (QL[i+1],QC[i],QR[i-1]);
  multiplying num/den by q0*q1*q2 gives products
    P_RL[m]=QR[m]*QL[m+2], P_RC[m]=QR[m]*QC[m+1], P_CL[m]=QC[m]*QL[m+1]
  shared between um and up.  Candidate polynomials (cell-centric, /6):
    PA = U + d2/3 + 1.5G[m],  PAr = U + d2/3 - 1.5G[m-1]
    PB = U - d2/6 + 0.5G[m],  PBr = U - d2/6 - 0.5G[m-1]
  um(i) = 10*Nm*(1/Dm), Nm = .1*P_CL[i-1]*PA[i-2] + .6*P_RL[i-2]*PB[i-1]
                             + .3*P_RC[i-2]*PBr[i]
          Dm = .5*P_RC[i-2] + (P_CL[i-1]/6 + P_RL[i-2])   (=den/0.6)
  up(i) analogous with (P_RC[i-1],PAr[i+1]) / (P_RL[i-1],PBr[i]) /
  (P_CL[i],PB[i-1]).
  fhat(i) = 0.5*max(relu(um)^2, min(up,0)^2); flux fused as
    F'[i] = (50*ALPHA/DX) * max(sq(relu(Nm*rm)), sq(min(Np*rp,0)))
  FDM part carried at c2-scale: d2s = c2*d2, A3 = (c3/c2)*(d2s[j+1]-d2s[j-1])
  + d2s[j];  out[j] = (F'[j]-F'[j+1]) + A3[j].

EPS is raised from 1e-16 to 1e-6 inside the WENO weights only: the weights
are identical to fp32 rounding except on ~1e-6 of cells, where the induced
flux error is ~1e-6 relative to the (u_xxx-dominated) output. This keeps the
q-products inside fp32 dynamic range.
"""

import math

import numpy as np

import concourse.bass as bass
import concourse.bacc as bacc
import concourse.mybir as mybir
import concourse.tile as tile
from concourse import dve_ops
from concourse.bass_utils import run_bass_kernel_spmd
from concourse.dve_spec import (
    C0,
    C1,
    C2,
    Spec,
    Src0,
    Src1,
    Zero,
    lower,
    minn,
    relu,
    sq,
)
from concourse.dve_uop import DveOpSpec

# ---- problem constants -----------------------------------------------------
B, NX = 4096, 8192
N_CORES = 8
ROWS_PER_CORE = B // N_CORES  # 512
L = 16.0
DX = L / NX
ALPHA, BETA, GAMMA = 3.0, 0.1, 1.0
EPS_K = 1e-6  # WENO regulariser used on-chip (reference uses 1e-16; see above)
C13 = 13.0 / 12.0
SQ_S = math.sqrt(1e3)  # sqrt of inner q-scale s
C2_FDM = BETA / DX / DX  # 26214.4
C3_FDM = -GAMMA / (2.0 * DX**3)  # -67108864.0
# um = Nm/(0.6*Dm) -> relu(um)^2 = relu(Nm*rm)^2/0.36; fhat = 0.5*max(...)
FLUXK = 0.5 * ALPHA / (0.36 * DX)  # scale on the fused max() flux terms

F32 = mybir.dt.float32
BF16 = mybir.dt.bfloat16
ADD = mybir.AluOpType.add
SUB = mybir.AluOpType.subtract
MUL = mybir.AluOpType.mult

# ---- custom fused DVE ops --------------------------------------------------
_REGISTERED = {}


def _register_dve(name, spec, subdim=False):
    """Register a custom DVE op in the dve_ops tables, computing its sha."""
    if name in _REGISTERED:
        return _REGISTERED[name]
    from concourse.dve_spec import _has_src1 as has_src1

    opcode = dve_ops._CUSTOM_DVE_ROW_BASE + len(dve_ops.OPS)
    shas = {}
    for ver in ("v3", "v4"):
        try:
            compiled = DveOpSpec(
                name=name,
                opcode=opcode,
                uops=lower(spec, ver=ver),
                rd1_en=has_src1(spec),
            )
            shas[ver] = compiled.sha(ver)
        except Exception:
            pass
    op = dve_ops.DveOp(name, spec, subdim=subdim, uops_sha=shas)
    dve_ops.OPS.append(op)
    dve_ops._SUB_OPCODE_FOR_NAME[name] = opcode
    dve_ops.CUSTOM_DVE_SPECS[name] = spec
    _REGISTERED[name] = op
    return op


def _q_specs():
    # scaled smoothness beta~ = s*beta, fused per flavour; the final
    # (beta~+eps~)^2 runs on the ScalarEngine as Square(x + eps~).
    # Src0 = G[m], Src1 = G[m-1].  (No Python literals in Spec bodies:
    # 3*S0-S1 == (S0-S1)+(S0+S0), S0-3*S1 == (S0-S1)-(S1+S1).)
    t = Src0 - Src1
    ca = sq(t * C0)  # c13*s*d2^2
    br = ca + sq((t + (Src0 + Src0)) * C1)
    bc = ca + sq((Src0 + Src1) * C1)
    bl = ca + sq((t - (Src1 + Src1)) * C1)
    return br, bc, bl


_BR_BODY, _BC_BODY, _BL_BODY = _q_specs()
OP_BR = _register_dve("ANT_WENO_BR", Spec(body=_BR_BODY))
OP_BC = _register_dve("ANT_WENO_BC", Spec(body=_BC_BODY))
OP_BL = _register_dve("ANT_WENO_BL", Spec(body=_BL_BODY))
# d2s = C0*(Src0-Src1)
OP_D2S = _register_dve("ANT_D2SCALE", Spec(body=(Src0 - Src1) * C0))
# C0*relu(Src0*Src1)^2  and  C0*min(Src0*Src1,0)^2
OP_RELSQ = _register_dve("ANT_RELSQS", Spec(body=sq(relu(Src0 * Src1)) * C0))
OP_MINSQ = _register_dve("ANT_MINSQS", Spec(body=sq(minn(Src0 * Src1, Zero)) * C0))


# ---- kernel body -----------------------------------------------------------
W = 2048  # spatial tile width (free axis)
# Total-order instruction chain: this walrus build rejects >1 sync wait on
# compute instructions; the chain guarantees exactly one.
LINEARIZE = False


# SBUF slot-reuse map: arrays whose live ranges are disjoint share a tag
# (same slots). Verified against the op order below.
_TAG = {
    "u": "u", "uh": "uh", "out": "out", "g": "g", "d2s": "d2s",
    "br": "t1", "n1": "t1", "n1p": "t1", "a2s": "t1",
    "bc": "t2", "n2": "t2", "n2p": "t2", "a1": "t2",
    "bl": "t3", "n12": "t3", "n12p": "t3",
    "qr": "qr", "n3": "qr", "n3p": "qr",
    "qc": "qc", "d1m": "qc", "d1p": "qc",
    "ql": "ql", "dm": "ql",
    "ta": "ta", "dp": "ta",
    "tb": "tb", "rm": "tb",
    "pa": "pa", "rp": "pa",
    "par": "par", "am": "par",
    "pb": "pb", "bm": "pb",
    "pbr": "pbr", "f": "pbr",
    "prl": "prl", "a3f": "prl",
    "prc": "prc", "pcl": "pcl", "nm": "g", "np": "np",
}


def _emit_tile(nc, pools, u_d, o_d, rb, ct):
    """Emit one [128 x W] output tile (row block rb, column tile ct)."""
    io_pool, pool = pools
    vec = nc.vector
    r0, r1 = rb * 128, (rb + 1) * 128
    c0 = ct * W
    WU = W + 6  # U halo width: columns map m = -3 .. W+2

    def t(key, width, dt=F32):
        tag = _TAG[key]
        p = io_pool if tag in ("u", "out") else pool
        return p.tile([128, width], dt, tag=tag, name=f"{key}_{rb}_{ct}")

    U = t("u", WU)
    # load with periodic wrap (halo 3 on both sides).  The TT ISA struct has
    # a single sync-wait slot, so a tile must not make its first consumer
    # wait on two DMAs: the small wrapped halo goes through a DVE copy (the
    # copy takes one DMA wait; program order on DVE covers it for the rest).
    lo, hi = c0 - 3, c0 + W + 3
    if lo < 0:
        Uh = t("uh", 3)
        nc.gpsimd.dma_start(Uh[:, :], u_d[r0:r1, NX + lo : NX])
        nc.gpsimd.dma_start(U[:, -lo : WU], u_d[r0:r1, 0 : hi])
        vec.tensor_copy(U[:, 0 : -lo], Uh[:, :])
    elif hi > NX:
        Uh = t("uh", 3)
        nc.gpsimd.dma_start(Uh[:, :], u_d[r0:r1, 0 : hi - NX])
        nc.gpsimd.dma_start(U[:, 0 : WU - (hi - NX)], u_d[r0:r1, lo:NX])
        vec.tensor_copy(U[:, WU - (hi - NX) : WU], Uh[:, :])
    else:
        nc.gpsimd.dma_start(U[:, :], u_d[r0:r1, lo:hi])

    # 01  G[m] = U[m+1]-U[m],  m = -3..W+1  (width W+5, col = m+3)
    G = t("g", W + 5)
    vec.tensor_sub(G[:, :], U[:, 1 : W + 6], U[:, 0 : W + 5])
    # 02  d2s[m] = c2*(G[m]-G[m-1]),  m = -2..W+1  (width W+4, col = m+2)
    d2s = t("d2s", W + 4)
    vec._custom_dve(
        OP_D2S, out=d2s[:, :], in0=G[:, 1 : W + 5], in1=G[:, 0 : W + 4], s0=C2_FDM
    )
    # 03-05  Q arrays, m = -2..W+1 (width W+4, col = m+2):
    # custom DVE computes beta~ = s*beta; ScalarE squares with +eps~ bias.
    qk0 = math.sqrt(C13) * SQ_S
    qk1 = 0.5 * SQ_S
    qk2 = EPS_K * 1e3  # eps~ = s*EPS_K
    QR = t("qr", W + 4, BF16)
    QC = t("qc", W + 4, BF16)
    QL = t("ql", W + 4, BF16)
    for op, dst, btag in ((OP_BR, QR, "br"), (OP_BC, QC, "bc"), (OP_BL, QL, "bl")):
        bt = t(btag, W + 4, BF16)
        vec._custom_dve(
            op,
            out=bt[:, :],
            in0=G[:, 1 : W + 5],
            in1=G[:, 0 : W + 4],
            s0=qk0,
            s1=qk1,
        )
        nc.scalar.activation(
            dst[:, :], bt[:, :], mybir.ActivationFunctionType.Square, bias=qk2
        )
    # 07  tA = U + d2s/(3 c2)   (m = -2..W+1); tB is redundant:
    # PB = tA + 0.5*G[m-1], PBr = tA - 0.5*G[m]  (identities via d2 = G-G[-1])
    tA = t("ta", W + 4)
    vec.scalar_tensor_tensor(
        tA[:, :], d2s[:, :], 1.0 / (3 * C2_FDM), U[:, 1 : W + 5], MUL, ADD
    )
    # 09-12  candidates (m = -2..W+1, col = m+2)
    PA = t("pa", W + 4)
    PAr = t("par", W + 4)
    PB = t("pb", W + 4)
    PBr = t("pbr", W + 4)
    vec.scalar_tensor_tensor(PA[:, :], G[:, 1 : W + 5], 1.5, tA[:, :], MUL, ADD)
    vec.scalar_tensor_tensor(PAr[:, :], G[:, 0 : W + 4], -1.5, tA[:, :], MUL, ADD)
    vec.scalar_tensor_tensor(PB[:, :], G[:, 0 : W + 4], 0.5, tA[:, :], MUL, ADD)
    vec.scalar_tensor_tensor(PBr[:, :], G[:, 1 : W + 5], -0.5, tA[:, :], MUL, ADD)
    # 13-15  q-products (col = m+2)
    PRL = t("prl", W + 2, BF16)  # m = -2..W-1
    PRC = t("prc", W + 3, BF16)  # m = -2..W
    PCL = t("pcl", W + 3, BF16)
    vec.tensor_mul(PRL[:, :], QR[:, 0 : W + 2], QL[:, 2 : W + 4])
    vec.tensor_mul(PRC[:, :], QR[:, 0 : W + 3], QC[:, 1 : W + 4])
    vec.tensor_mul(PCL[:, :], QC[:, 0 : W + 3], QL[:, 1 : W + 4])
    # interfaces i = 0..W (width W+1);  P_* col(m)=m+2, cand col(m)=m+2
    WI = W + 1
    n1 = t("n1", WI, BF16)
    n2 = t("n2", WI, BF16)
    n12 = t("n12", WI, BF16)
    n3 = t("n3", WI, BF16)
    Nm = t("nm", WI, BF16)
    vec.scalar_tensor_tensor(n1[:, :], PCL[:, 1 : WI + 1], 0.1, PA[:, 0:WI], MUL, MUL)
    vec.scalar_tensor_tensor(n2[:, :], PRL[:, 0:WI], 0.6, PB[:, 1 : WI + 1], MUL, MUL)
    vec.tensor_add(n12[:, :], n1[:, :], n2[:, :])
    vec.scalar_tensor_tensor(n3[:, :], PRC[:, 0:WI], 0.3, PBr[:, 2 : WI + 2], MUL, MUL)
    vec.tensor_add(Nm[:, :], n12[:, :], n3[:, :])
    d1m = t("d1m", WI)
    Dm = t("dm", WI)
    vec.scalar_tensor_tensor(
        d1m[:, :], PCL[:, 1 : WI + 1], 1.0 / 6.0, PRL[:, 0:WI], MUL, ADD
    )
    vec.scalar_tensor_tensor(Dm[:, :], PRC[:, 0:WI], 0.5, d1m[:, :], MUL, ADD)
    n1p = t("n1p", WI, BF16)
    n2p = t("n2p", WI, BF16)
    n12p = t("n12p", WI, BF16)
    n3p = t("n3p", WI, BF16)
    Np = t("np", WI, BF16)
    vec.scalar_tensor_tensor(
        n1p[:, :], PRC[:, 1 : WI + 1], 0.1, PAr[:, 3 : WI + 3], MUL, MUL
    )
    vec.scalar_tensor_tensor(
        n2p[:, :], PRL[:, 1 : WI + 1], 0.6, PBr[:, 2 : WI + 2], MUL, MUL
    )
    vec.tensor_add(n12p[:, :], n1p[:, :], n2p[:, :])
    vec.scalar_tensor_tensor(
        n3p[:, :], PCL[:, 2 : WI + 2], 0.3, PB[:, 1 : WI + 1], MUL, MUL
    )
    vec.tensor_add(Np[:, :], n12p[:, :], n3p[:, :])
    d1p = t("d1p", WI)
    Dp = t("dp", WI)
    vec.scalar_tensor_tensor(
        d1p[:, :], PRC[:, 1 : WI + 1], 1.0 / 6.0, PRL[:, 1 : WI + 1], MUL, ADD
    )
    vec.scalar_tensor_tensor(Dp[:, :], PCL[:, 2 : WI + 2], 0.5, d1p[:, :], MUL, ADD)
    # 30-31 reciprocals (approx, ~18 bits — weight normalisation only)
    rm = t("rm", WI)
    rp = t("rp", WI)
    vec.reciprocal_approx_fast(out=rm[:, :], in_=Dm[:, :])
    vec.reciprocal_approx_fast(out=rp[:, :], in_=Dp[:, :])
    # 32-33 fused flux halves: FLUXK/100 * relu(10*Nm*rm)^2 etc.
    AM = t("am", WI, BF16)
    BM = t("bm", WI, BF16)
    vec._custom_dve(OP_RELSQ, out=AM[:, :], in0=Nm[:, :], in1=rm[:, :], s0=FLUXK)
    vec._custom_dve(OP_MINSQ, out=BM[:, :], in0=Np[:, :], in1=rp[:, :], s0=FLUXK)
    # 34 F'[i] = max(AM,BM)
    F = t("f", WI, BF16)
    vec.tensor_max(F[:, :], AM[:, :], BM[:, :])
    # FDM tail (output cells j = 0..W-1)
    A2s = t("a2s", W)
    A3f = t("a3f", W)
    A1 = t("a1", W)
    OUT = t("out", W)
    vec.tensor_sub(A2s[:, :], d2s[:, 3 : W + 3], d2s[:, 1 : W + 1])
    vec.scalar_tensor_tensor(
        A3f[:, :], A2s[:, :], C3_FDM / C2_FDM, d2s[:, 2 : W + 2], MUL, ADD
    )
    vec.tensor_sub(A1[:, :], F[:, 0:W], F[:, 1 : W + 1])
    vec.tensor_add(OUT[:, :], A1[:, :], A3f[:, :])
    nc.gpsimd.dma_start(o_d[r0:r1, c0 : c0 + W], OUT[:, :])


def _build_nc():
    nc = bacc.Bacc("TRN2", target_bir_lowering=False, debug=False)
    # const AP for the ScalarE Square bias (eps~), same pattern as Bass init
    eps_val = EPS_K * 1e3
    eps_t = nc.alloc_sbuf_tensor("const-float32-weno-eps", [128, 1], F32)
    nc.gpsimd.memset(eps_t.ap(), eps_val)
    nc.const_aps.aps[(F32, eps_val)] = eps_t.ap()
    nc.all_engine_barrier()
    u_d = nc.dram_tensor("u", [ROWS_PER_CORE, NX], F32, kind="ExternalInput")
    o_d = nc.dram_tensor("out", [ROWS_PER_CORE, NX], F32, kind="ExternalOutput")
    with tile.TileContext(nc, linearize=LINEARIZE) as tc:
        with (
            tc.tile_pool(name="io", bufs=2) as io_pool,
            tc.tile_pool(name="main", bufs=1) as pool,
        ):
            for rb in range(ROWS_PER_CORE // 128):
                for ct in range(NX // W):
                    _emit_tile(nc, (io_pool, pool), u_d, o_d, rb, ct)
    nc.compile()
    return nc


_NC = None


def _get_nc():
    global _NC
    if _NC is None:
        _NC = _build_nc()
    return _NC


def _execute(u, trace=False):
    nc = _get_nc()
    u = np.ascontiguousarray(np.asarray(u, dtype=np.float32))
    in_maps = [
        {"u": u[i * ROWS_PER_CORE : (i + 1) * ROWS_PER_CORE]} for i in range(N_CORES)
    ]
    res = run_bass_kernel_spmd(nc, in_maps, list(range(N_CORES)), trace=trace)
    out = np.concatenate([res.results[i]["out"] for i in range(N_CORES)], axis=0)
    return out, res


def kernel(u, t=None, **_ignored):
    out, _ = _execute(u, trace=False)
    return out
